# revision 23
# baseline (speedup 1.0000x reference)
"""GroupConvTranspose3d (kernel 2, stride 2) Trainium2 Bass kernel.

Math: y[b,g,o,2d+i,2h+j,2w+k] = sum_c x[b,g,c,d,h,w] * K[c,o,i,j,k]
(all 16 groups share the same kernel). Shapes are hardcoded:
  x: (2,16,128,16,16,16) f32, kernel: (128,128,2,2,2) f32
  y: (2,16,128,32,32,32) f32

Strategy: data-parallel over the 32 (b,g) pairs, 4 per NeuronCore.
The kernel is HBM-store-bound (~67 MB of f32 output per core), so the
structure keeps the store DMA queue saturated end to end:

- x and K are cast to bf16 on the host (halves HBM read traffic and
  PE-side SBUF reads; rounding error ~1e-3 rel, well inside the 2e-2
  tolerance). K is pre-shuffled to tap-major [c, (t, o)] so the 8 taps
  are contiguous SBUF slices needing no extraction copies.
- ALL x loads (4 MB bf16/core) are prefetched up front: K + pair-0
  chunks on the sync HWDGE queue (fast start), pairs 1-3 on the
  otherwise idle GPSIMD SWDGE queue. They complete inside the pipeline
  ramp, so the steady state has zero load/store DMA contention.
- Per d-pair (512 x-cols): 8 matmuls out[o,(d2,h,w)=512] =
  K_t[c,o].T @ x[c,512] in bf16, landing pairwise in 2-bank PSUM
  tiles; 4 batched strided PSUM->SBUF copies (vector/scalar split)
  realize the (d,i),(h,j),(w,k) interleave into an [o=128, 4096] f32
  slab, stored from the sync HWDGE queue as 16KB-contiguous-per-
  partition runs. The very first d-pair runs as two 256-col halves
  with 1MB stores to cut the time-to-first-store.
"""

import sys

if "/opt/trn_rl_repo" not in sys.path:
    sys.path.insert(0, "/opt/trn_rl_repo")

import numpy as np

B, G, CIN, COUT, D, H, W = 2, 16, 128, 128, 16, 16, 16
NCORES = 8
PAIRS_PER_CORE = (B * G) // NCORES  # 4
DHW = D * H * W  # 4096
OUT_SPATIAL = 8 * DHW  # 32768 per (b,g,o)
NDP = D // 2  # 8 d-pairs per (b,g)

_CACHE = {}


def _build_program(oslab_bufs=4, load_eng="gpsimd", store_eng="sync",
                   first_pair_chunks=8, warm_mms=0, head_on_store=True,
                   drain_pairs=False, split_first_dp=True):
    import concourse.mybir as mybir
    import concourse.tile as tile
    from concourse import bacc
    from concourse.bass import ds

    f32 = mybir.dt.float32
    bf16 = mybir.dt.bfloat16

    nc = bacc.Bacc(None, target_bir_lowering=False)
    x_d = nc.declare_dram_parameter("x", [PAIRS_PER_CORE, CIN, DHW], bf16, isOutput=False)
    k_d = nc.declare_dram_parameter("kernel", [CIN, 8 * COUT], bf16, isOutput=False)
    y_d = nc.declare_dram_parameter("y", [PAIRS_PER_CORE, COUT, OUT_SPATIAL], f32, isOutput=True)

    CHUNK = DHW // first_pair_chunks

    with tile.TileContext(nc) as tc:
        with (
            tc.tile_pool(name="ktap", bufs=1) as ktap_pool,
            tc.tile_pool(name="xchunk", bufs=first_pair_chunks) as xc_pool,
            tc.tile_pool(name="xslab", bufs=PAIRS_PER_CORE - 1) as x_pool,
            tc.tile_pool(name="oslab", bufs=oslab_bufs) as out_pool,
            tc.tile_pool(name="psum", bufs=4 if drain_pairs else 8, space="PSUM") as psum_pool,
        ):
            ld = getattr(nc, load_eng)
            st = getattr(nc, store_eng)

            # K arrives tap-major [c, (t, o)] bf16; each tap is a
            # contiguous [c, o] slice usable directly as matmul lhsT.
            # K + pair-0 chunks ride the fast HWDGE store queue (idle at
            # t=0, ~0.6us issue each) so the first matmul fires ~2us in;
            # pairs 1..3 prefetch concurrently on the GPSIMD SWDGE
            # queue, completing during pipeline ramp so the steady state
            # has zero load/store DMA contention.
            hd = st if head_on_store else ld
            kraw = ktap_pool.tile([CIN, 8 * COUT], bf16)
            hd.dma_start(out=kraw[:], in_=k_d[:])
            ktaps = [kraw[:, ds(t * COUT, COUT)] for t in range(8)]

            # Optional PE warm-up (default off: traces show the f32
            # dummy matmuls double-pump LOW/HIGH and occupy the in-order
            # PE past the point where real data is ready, delaying the
            # first store more than the p-state ramp saves).
            if warm_mms:
                wt = ktap_pool.tile([CIN, COUT], f32, tag="warm")
                nc.vector.memset(wt[:], 0.0)
                for _ in range(warm_mms):
                    wps = psum_pool.tile([COUT, COUT], f32, tag="ps")
                    nc.tensor.matmul(wps[:], wt[:], wt[:], start=True, stop=True)

            # Single tag per pool (bufs = live-tile count): distinct tags
            # would each allocate their own semaphore stream, and every
            # allocated semaphore costs ~160ns in the framework's
            # serial per-engine teardown reset storm (~8us total).
            chunk0 = []
            for ci in range(first_pair_chunks):
                xs = xc_pool.tile([CIN, CHUNK], bf16, tag="x0c")
                hd.dma_start(out=xs[:], in_=x_d[0, :, ds(ci * CHUNK, CHUNK)])
                chunk0.append(xs)
            slabs = []
            for bgi in range(1, PAIRS_PER_CORE):
                xs = x_pool.tile([CIN, DHW], bf16, tag="xslab")
                ld.dma_start(out=xs[:], in_=x_d[bgi, :, :])
                slabs.append(xs)

            # Interleave vector/scalar tap drains so both PSUM-drain
            # engines start as soon as their first matmul lands.
            TAP_ORDER = (0, 4, 1, 5, 2, 6, 3, 7)
            VEC_TAPS = {0, 1, 2, 3}
            CPD = 512 // CHUNK if CHUNK < 512 else 1  # chunks per d-pair

            for bgi in range(PAIRS_PER_CORE):
                for dp in range(NDP):
                    oslab = out_pool.tile([COUT, 4096], f32)
                    ov = oslab[:].rearrange(
                        "p (dl i h j w k) -> p dl i h j w k",
                        dl=2, i=2, h=16, j=2, w=16, k=2,
                    )
                    if bgi == 0:
                        rhs = chunk0[(dp * 512) // CHUNK][
                            :, ds((dp * 512) % CHUNK, 512)
                        ]
                    else:
                        rhs = slabs[bgi - 1][:, ds(dp * 512, 512)]
                    if split_first_dp and bgi == 0 and dp == 0:
                        # First d-pair runs as two dl-halves of 256 cols
                        # with 1MB stores so the first store launches
                        # ~3.5us earlier (the head is store-latency
                        # critical while the pipe fills). dl=0/1 halves
                        # are contiguous in both x and y.
                        for dl in range(2):
                            for t in TAP_ORDER:
                                ps = psum_pool.tile([COUT, 256], f32, tag="ps")
                                nc.tensor.matmul(
                                    ps[:], ktaps[t],
                                    chunk0[0][:, ds(dl * 256, 256)],
                                    start=True, stop=True,
                                )
                                i, j, k = (t >> 2) & 1, (t >> 1) & 1, t & 1
                                src = ps[:].rearrange(
                                    "p (h w) -> p h w", h=16, w=16
                                )
                                dst = ov[:, dl, i, :, j, :, k]
                                if t in VEC_TAPS:
                                    nc.vector.tensor_copy(dst, src)
                                else:
                                    nc.scalar.copy(dst, src)
                            st.dma_start(
                                out=y_d[0, :, ds(dl * 2048, 2048)],
                                in_=oslab[:, ds(dl * 2048, 2048)],
                            )
                        continue
                    if drain_pairs:
                        # Batched drains: each (i,j) tap pair lands in one
                        # 2-bank PSUM tile and drains with a single 1024-el
                        # copy (4-dim AP), halving drain instruction count.
                        # DEFAULT OFF: two matmuls feeding slices of one
                        # PSUM tile makes walrus allocate ~200 extra
                        # semaphores, and its serial per-engine teardown
                        # reset storm grows from ~3.8us to ~9.2us — more
                        # than the drains save.
                        for ta, tb in ((0, 1), (4, 5), (2, 3), (6, 7)):
                            ps2 = psum_pool.tile([COUT, 1024], f32, tag="ps")
                            nc.tensor.matmul(
                                ps2[:, ds(0, 512)], ktaps[ta], rhs,
                                start=True, stop=True,
                            )
                            nc.tensor.matmul(
                                ps2[:, ds(512, 512)], ktaps[tb], rhs,
                                start=True, stop=True,
                            )
                            i, j = (ta >> 2) & 1, (ta >> 1) & 1
                            src = ps2[:].rearrange(
                                "p (k dl h w) -> p dl h w k",
                                k=2, dl=2, h=16, w=16,
                            )
                            dst = ov[:, :, i, :, j, :, :]
                            if i == 0:
                                nc.vector.tensor_copy(dst, src)
                            else:
                                nc.scalar.copy(dst, src)
                    else:
                        for t in TAP_ORDER:
                            ps = psum_pool.tile([COUT, 512], f32, tag="ps")
                            nc.tensor.matmul(
                                ps[:], ktaps[t], rhs,
                                start=True, stop=True,
                            )
                            i, j, k = (t >> 2) & 1, (t >> 1) & 1, t & 1
                            src = ps[:].rearrange(
                                "p (dl h w) -> p dl h w", dl=2, h=16, w=16
                            )
                            dst = ov[:, :, i, :, j, :, k]
                            if t in VEC_TAPS:
                                nc.vector.tensor_copy(dst, src)
                            else:
                                nc.scalar.copy(dst, src)
                    st.dma_start(
                        out=y_d[bgi, :, ds(dp * 4096, 4096)],
                        in_=oslab[:],
                    )
    nc.compile()
    return nc


def _get_program(**kw):
    key = tuple(sorted(kw.items()))
    if key not in _CACHE:
        _CACHE[key] = _build_program(**kw)
    return _CACHE[key]


def _make_in_maps(x, kernel):
    import ml_dtypes

    xr = np.ascontiguousarray(
        np.asarray(x, dtype=np.float32).reshape(B * G, CIN, DHW)
    ).astype(ml_dtypes.bfloat16)
    # (c, o, i, j, k) -> tap-major (c, (t=ijk), o)
    kr = np.ascontiguousarray(
        np.asarray(kernel, dtype=np.float32)
        .reshape(CIN, COUT, 8)
        .transpose(0, 2, 1)
        .reshape(CIN, 8 * COUT)
    ).astype(ml_dtypes.bfloat16)
    return [
        {"x": xr[i * PAIRS_PER_CORE : (i + 1) * PAIRS_PER_CORE], "kernel": kr}
        for i in range(NCORES)
    ]


def _gather(results):
    y = np.concatenate([results[i]["y"] for i in range(NCORES)], axis=0)
    return y.reshape(B, G, COUT, 2 * D, 2 * H, 2 * W)


def run(x, kernel, trace=False, build_kw=None, **kw):
    """Run on hardware; returns (y, BassKernelResults)."""
    import json
    import os

    from concourse.bass_utils import run_bass_kernel_spmd

    if build_kw is None and os.environ.get("KERNEL_BUILD_KW"):
        build_kw = json.loads(os.environ["KERNEL_BUILD_KW"])
    nc = _get_program(**(build_kw or {}))
    res = run_bass_kernel_spmd(
        nc, _make_in_maps(x, kernel), list(range(NCORES)), trace=trace, **kw
    )
    return _gather(res.results), res


def kernel(**inputs):
    y, _ = run(inputs["x"], inputs["kernel"])
    return y
